# revision 40
# baseline (speedup 1.0000x reference)
"""Trainium2 Bass kernel for FOAM embedding (GNN message passing).

Strategy (8 NeuronCores, SPMD, no collectives):
  - Edges are sorted by edge_src. Host partitions nodes into 8 contiguous
    ranges with balanced edge counts; each core owns its nodes' edges.
  - Within a core, nodes are packed greedily into "blocks" of <=128 edges
    and <=7 node slots. Each block's 128 edge slots sit on the 128 SBUF
    partitions.
  - The segment-sum over edges becomes one PE matmul per block:
        lhsT = Dij [128e x 128b]   (stationary)
        rhs  = S   [128e x 70]     S[e, l*10+m] = ohw[e,l,m] * Y[e, m]
    where ohw folds the slot one-hot, the SH constants k_m and the
    per-edge switch factor sqrt(2/rc)*switch/d (host-side constants /
    trivial input scalings).  This gives PSUM [128b x (slot, m)] = rhoi
    for up to 7 nodes at once.
  - Phase 3 contracts rhoi with the (row-permuted) Dense weights over the
    128 basis dim on the PE; xl/yl land in one two-bank PSUM tile, one
    copy to SBUF, then a bf16 2x multiply + strided reduce for
    (xl*yl).sum(m).
  - Host reassembles the full [15000, 528] output (species enc columns
    are a pure table gather, done on host).
"""

import os
import sys

import numpy as np

for _p in ("/opt/trn_rl_repo", "/root/.axon_site/_ro/trn_rl_repo"):
    if os.path.isdir(_p) and _p not in sys.path:
        sys.path.insert(0, _p)

import ml_dtypes  # noqa: E402

# ---------------- problem constants (hardcoded per spec) ----------------
N_RADIAL = 8
N_SPEC = 16
ZMAX = 64
CUTOFF = 5.0
NCHAN = 128
NB = N_RADIAL * N_SPEC  # 128 basis
M9 = 9                  # real SH components up to l=2
M10 = 10                # padded (plane 9 is zero)

NCORES = 8
P = 128                 # edges per block == partitions
NSLOT = 7               # node slots per block
SCOLS = NSLOT * M10     # 70 moving columns per block
CH = 56                 # blocks per chunk
PSG = 7                 # blocks per PSUM scatter tile (7*70=490 <= 512)

BF16 = ml_dtypes.bfloat16

_COMPILED = {}
TRACE = False          # set True to capture an NTFF profile
LAST_RESULT = None     # BassKernelResults of the last kernel() call

# internal SH plane order (l-groups contiguous; order within group is free):
#   m0: 1, m1..3: x,y,z, m4: xy, m5: yz, m6: xz, m7: 2z^2-x^2-y^2,
#   m8: x^2-y^2, m9: zero pad
_S5, _S15 = 5.0 ** 0.5, 15.0 ** 0.5
KM = np.array([1.0, 3.0 ** 0.5, 3.0 ** 0.5, 3.0 ** 0.5,
               _S15, _S15, _S15, 0.5 * _S5, 0.5 * _S15, 0.0], np.float32)


# ======================= host-side preprocessing =======================

def _partition_and_pack(edge_src, n_nodes):
    """Split nodes into NCORES contiguous ranges (edge balanced), then pack
    nodes into blocks of <=P edges / <=NSLOT nodes per core."""
    es = np.asarray(edge_src, dtype=np.int64)
    E = es.shape[0]
    deg = np.bincount(es, minlength=n_nodes)
    splits = [0]
    for c in range(1, NCORES):
        n = int(es[min((c * E) // NCORES, E - 1)])
        n = max(n, splits[-1])
        splits.append(n)
    splits.append(n_nodes)

    cores = []
    for c in range(NCORES):
        nlo, nhi = splits[c], splits[c + 1]
        blocks = []
        n = nlo
        while n < nhi:
            cnt = 0
            esum = 0
            while (n + cnt < nhi and cnt < NSLOT
                   and esum + deg[n + cnt] <= P):
                esum += deg[n + cnt]
                cnt += 1
            if cnt == 0:
                raise ValueError(
                    f"node {n} has degree {deg[n]} > {P}; unsupported")
            blocks.append((n, cnt, esum))
            n += cnt
        cores.append({"nlo": nlo, "nhi": nhi, "blocks": blocks})
    return cores, deg


def _build_host_inputs(inputs, cores, deg, B, nchunk):
    """Build per-core DRAM input arrays in the device layout."""
    dist = np.asarray(inputs["distances"], np.float32)
    vec = np.asarray(inputs["vec"], np.float32)
    switch = np.asarray(inputs["switch"], np.float32)
    st = np.asarray(inputs["species_table"], np.float32)
    species = np.asarray(inputs["species"], np.int64)
    esrc = np.asarray(inputs["edge_src"], np.int64)
    edst = np.asarray(inputs["edge_dst"], np.int64)
    N_NODES = species.shape[0]

    senc_node = st[species]          # [N, 16]
    first_edge = np.searchsorted(esrc, np.arange(N_NODES + 1), side="left")
    bess = (2.0 / CUTOFF) ** 0.5
    swf = bess * switch / dist       # per-edge switch factor (folded in ohw)

    per_core = []
    for c in range(NCORES):
        blocks = cores[c]["blocks"]
        nb = len(blocks)
        edf = np.zeros((B, 4, P), np.float32)
        edf[:, 0, :] = 1.0                      # dist pad
        edf[:, 1, :] = 1.0                      # vx pad
        senc_e = np.zeros((B, P, N_SPEC), np.float32)
        ohw = np.zeros((B, P, NSLOT), np.float32)
        slot_node = np.full((B * NSLOT,), -1, np.int64)

        for k, (n0, cnt, esum) in enumerate(blocks):
            e0 = first_edge[n0]
            e1 = first_edge[n0 + cnt]
            idx = np.arange(e0, e1)
            p = idx - e0
            edf[k, 0, p] = dist[idx]
            edf[k, 1, p] = vec[idx, 0]
            edf[k, 2, p] = vec[idx, 1]
            edf[k, 3, p] = vec[idx, 2]
            senc_e[k, p, :] = senc_node[edst[idx]]
            loc = esrc[idx] - n0
            ohw[k, p, loc] = swf[idx]
            slot_node[k * NSLOT: k * NSLOT + cnt] = np.arange(n0, n0 + cnt)

        # edf planes, whole core: [128, 4, B]
        edf_dev = np.ascontiguousarray(edf.transpose(2, 1, 0))
        # senc_rep[p, c, s, r] = senc[p, c, s]
        senc_dev = np.ascontiguousarray(
            np.repeat(
                senc_e.reshape(nchunk, CH, P, N_SPEC).transpose(0, 2, 1, 3),
                N_RADIAL, axis=3,
            )
        ).astype(BF16)
        # ohw_rep[p, c, l, m] = oh[p, c, l] * swf[e] * km[m]
        oh_dev = np.ascontiguousarray(
            np.repeat(
                ohw.reshape(nchunk, CH, P, NSLOT).transpose(0, 2, 1, 3),
                M10, axis=3,
            ).reshape(nchunk, P, CH, NSLOT, M10) * KM
        ).astype(BF16)

        per_core.append(
            {
                "edf": edf_dev.reshape(P, 4 * B),
                "senc": senc_dev.reshape(nchunk, P, CH * NB),
                "oh": oh_dev.reshape(nchunk, P, CH * SCOLS),
                "slot_node": slot_node,
                "nblocks": nb,
            }
        )
    return per_core


def _perm_w(W):
    """Permute Dense weight rows from rs-order (r*16+s) to (s*8+r) order."""
    W = np.asarray(W, np.float32)
    return np.ascontiguousarray(
        W.reshape(N_RADIAL, N_SPEC, -1).transpose(1, 0, 2).reshape(NB, -1)
    )


# ========================= device program =========================

def _build_program(nchunk):
    import concourse.bacc as bacc
    import concourse.mybir as mybir
    import concourse.tile as tile
    from concourse.alu_op_type import AluOpType as alu

    fp32 = mybir.dt.float32
    bf16 = mybir.dt.bfloat16

    B = nchunk * CH
    NS = NSLOT * B

    nc = bacc.Bacc("TRN2", target_bir_lowering=False, debug=False,
                   num_devices=NCORES)

    edf_d = nc.dram_tensor("edf", [P, 4 * B], fp32, kind="ExternalInput")
    senc_d = nc.dram_tensor("senc", [nchunk, P, CH * NB], bf16,
                            kind="ExternalInput")
    oh_d = nc.dram_tensor("oh", [nchunk, P, CH * SCOLS], bf16,
                          kind="ExternalInput")
    wx_d = nc.dram_tensor("wx", [P, 3 * NCHAN], bf16, kind="ExternalInput")
    wy_d = nc.dram_tensor("wy", [P, 3 * NCHAN], bf16, kind="ExternalInput")
    rhoi0_d = nc.dram_tensor("rhoi0", [P, NS], bf16, kind="ExternalOutput")
    xy_d = nc.dram_tensor("xy", [P, 3 * NS], fp32, kind="ExternalOutput")

    with tile.TileContext(nc) as tc:
        with (
            tc.tile_pool(name="const", bufs=1) as cpool,
            tc.tile_pool(name="pha", bufs=1) as papool,
            tc.tile_pool(name="chunk", bufs=2) as ckpool,
            tc.tile_pool(name="big", bufs=1) as bigpool,
            tc.tile_pool(name="ps_sc", bufs=4, space="PSUM") as pssc,
            tc.tile_pool(name="ps_xy", bufs=2, space="PSUM") as psxy,
        ):
            wx = cpool.tile([P, 3 * NCHAN], bf16, tag="wx")
            wy = cpool.tile([P, 3 * NCHAN], bf16, tag="wy")
            nc.sync.dma_start(out=wx[:], in_=wx_d[:])
            nc.sync.dma_start(out=wy[:], in_=wy_d[:])
            half_pi = cpool.tile([P, 1], fp32, tag="halfpi")
            nc.vector.memset(half_pi[:], float(np.pi / 2))

            rhoi_sb = bigpool.tile([P, M10 * NS], bf16, tag="rhoi")

            # ============ phase A: per-edge scalars, whole core ============
            edf = papool.tile([P, 4 * B], fp32, tag="edf")
            nc.sync.dma_start(out=edf[:], in_=edf_d[:])
            d_ap = edf[:, 0:B]
            v_ap = edf[:, B:4 * B]

            rinv = papool.tile([P, B], fp32, tag="rinv")
            nc.vector.reciprocal(out=rinv[:], in_=d_ap)
            u = papool.tile([P, 3 * B], fp32, tag="u")
            nc.vector.tensor_tensor(
                out=u[:].rearrange("p (t c) -> p t c", t=3),
                in0=v_ap.rearrange("p (t c) -> p t c", t=3),
                in1=rinv[:].unsqueeze(1).broadcast_to([P, 3, B]),
                op=alu.mult,
            )
            ux, uy, uz = (u[:, i * B:(i + 1) * B] for i in range(3))

            # radial: rbp[p, n, c] = sin((n+1) theta), theta = pi d / rc,
            # via Chebyshev recurrence (ACT Sin valid on [-4.18, 4.18]).
            # Built on contiguous n-major planes (strided writes are slow),
            # then one transposing cast to r-innermost bf16.
            rbp = papool.tile([P, N_RADIAL * B], fp32, tag="rbp")
            cos2 = papool.tile([P, B], fp32, tag="cos2")
            nc.scalar.activation(
                out=rbp[:, 0:B], in_=d_ap,
                func=mybir.ActivationFunctionType.Sin,
                scale=float(np.pi / CUTOFF),
            )
            nc.scalar.activation(
                out=cos2[:], in_=d_ap,
                func=mybir.ActivationFunctionType.Sin,
                scale=float(-np.pi / CUTOFF), bias=half_pi[:],
            )
            nc.vector.tensor_scalar(
                out=cos2[:], in0=cos2[:], scalar1=2.0, scalar2=None,
                op0=alu.mult,
            )
            nc.vector.tensor_tensor(
                out=rbp[:, B:2 * B], in0=cos2[:], in1=rbp[:, 0:B],
                op=alu.mult)
            for n in range(2, N_RADIAL):
                nc.vector.tensor_tensor(
                    out=rbp[:, n * B:(n + 1) * B], in0=cos2[:],
                    in1=rbp[:, (n - 1) * B:n * B], op=alu.mult)
                nc.vector.tensor_tensor(
                    out=rbp[:, n * B:(n + 1) * B],
                    in0=rbp[:, n * B:(n + 1) * B],
                    in1=rbp[:, (n - 2) * B:(n - 1) * B], op=alu.subtract)
            rb_t = papool.tile([P, B * N_RADIAL], bf16, tag="rbt")
            nc.vector.tensor_copy(
                out=rb_t[:].rearrange("p (c n) -> p c n", n=N_RADIAL),
                in_=rbp[:].rearrange("p (n c) -> p n c", n=N_RADIAL)
                    .transpose([0, 2, 1]),
            )

            # Y planes, m-major fp32 (contiguous builds), then one
            # transposing cast to m-innermost bf16 for the S broadcast.
            Yp = papool.tile([P, M10 * B], fp32, tag="Yp")
            nc.vector.memset(Yp[:, 0:B], 1.0)
            nc.vector.memset(Yp[:, 9 * B:10 * B], 0.0)
            nc.vector.tensor_copy(out=Yp[:, B:4 * B], in_=u[:])
            # m4 = x*y, m5 = y*z  (pair op), m6 = x*z
            nc.vector.tensor_tensor(
                out=Yp[:, 4 * B:6 * B], in0=u[:, 0:2 * B],
                in1=u[:, B:3 * B], op=alu.mult)
            nc.vector.tensor_tensor(
                out=Yp[:, 6 * B:7 * B], in0=ux, in1=uz, op=alu.mult)
            # m7 = 2 z^2 - x^2 - y^2, m8 = x^2 - y^2   (|u| = 1)
            sq = papool.tile([P, 3 * B], fp32, tag="sq")
            nc.vector.tensor_tensor(out=sq[:], in0=u[:], in1=u[:],
                                    op=alu.mult)
            ab = papool.tile([P, B], fp32, tag="ab")
            nc.vector.tensor_tensor(
                out=ab[:], in0=sq[:, 0:B], in1=sq[:, B:2 * B], op=alu.add)
            nc.vector.scalar_tensor_tensor(
                out=Yp[:, 7 * B:8 * B], in0=sq[:, 2 * B:3 * B], scalar=2.0,
                in1=ab[:], op0=alu.mult, op1=alu.subtract)
            nc.vector.tensor_tensor(
                out=Yp[:, 8 * B:9 * B], in0=sq[:, 0:B], in1=sq[:, B:2 * B],
                op=alu.subtract)
            Y = papool.tile([P, B * M10], bf16, tag="Y")
            nc.vector.tensor_copy(
                out=Y[:].rearrange("p (c m) -> p c m", m=M10),
                in_=Yp[:].rearrange("p (m c) -> p m c", m=M10)
                    .transpose([0, 2, 1]),
            )

            # ================= per-chunk scatter + phase 3 =================
            ncopy = 0
            nxcopy = 0
            for ci in range(nchunk):
                senc = ckpool.tile([P, CH * NB], bf16, tag="senc")
                oh = ckpool.tile([P, CH * SCOLS], bf16, tag="oh")
                nc.sync.dma_start(out=senc[:], in_=senc_d[ci])
                nc.sync.dma_start(out=oh[:], in_=oh_d[ci])

                # S[p, blk, l*10+m] = ohw[p, blk, l, m] * Y[p, blk, m]
                S = ckpool.tile([P, CH * SCOLS], bf16, tag="S")
                nc.gpsimd.tensor_tensor(
                    out=S[:].rearrange("p (c l m) -> p c l m",
                                       l=NSLOT, m=M10),
                    in0=oh[:].rearrange("p (c l m) -> p c l m",
                                        l=NSLOT, m=M10),
                    in1=Y[:, ci * CH * M10:(ci + 1) * CH * M10]
                        .rearrange("p (c m) -> p c m", m=M10)
                        .unsqueeze(2).broadcast_to([P, CH, NSLOT, M10]),
                    op=alu.mult,
                )
                # Dij[p, blk, s*8+r] = senc_rep[p, blk, s, r] * rb_t[p, blk, r]
                Dij = ckpool.tile([P, CH * NB], bf16, tag="Dij")
                nc.vector.tensor_tensor(
                    out=Dij[:].rearrange("p (c s r) -> p c s r",
                                         s=N_SPEC, r=N_RADIAL),
                    in0=senc[:].rearrange("p (c s r) -> p c s r",
                                          s=N_SPEC, r=N_RADIAL),
                    in1=rb_t[:, ci * CH * N_RADIAL:(ci + 1) * CH * N_RADIAL]
                        .rearrange("p (c r) -> p c r", r=N_RADIAL)
                        .unsqueeze(2).broadcast_to([P, CH, N_SPEC, N_RADIAL]),
                    op=alu.mult,
                )

                # scatter matmuls: PSG blocks per PSUM tile, then one
                # contiguous copy into slot-major rhoi_sb (col = slot*10+m)
                for g in range(CH // PSG):
                    pst = pssc.tile([P, PSG * SCOLS], fp32, tag="psc")
                    for j in range(PSG):
                        k = g * PSG + j
                        nc.tensor.matmul(
                            out=pst[:, j * SCOLS:(j + 1) * SCOLS],
                            lhsT=Dij[:, k * NB:(k + 1) * NB],
                            rhs=S[:, k * SCOLS:(k + 1) * SCOLS],
                            start=True, stop=True,
                        )
                    col0 = (ci * CH + g * PSG) * NSLOT * M10
                    dst = rhoi_sb[:, col0:col0 + PSG * SCOLS]
                    nc.scalar.copy(out=dst, in_=pst[:])
                    ncopy += 1

                # ---- phase 3, interleaved per chunk ----
                slotc = CH * NSLOT
                base = ci * slotc
                for l in range(3):
                    mg = 2 * l + 1
                    m0 = l * l
                    nsl = -(-slotc // (512 // mg))
                    ssz0 = -(-slotc // nsl)
                    wxl = wx[:, l * NCHAN:(l + 1) * NCHAN]
                    wyl = wy[:, l * NCHAN:(l + 1) * NCHAN]
                    for t in range(nsl):
                        s0 = base + t * ssz0
                        ssz = min(ssz0, base + slotc - s0)
                        cols = ssz * mg
                        mov = rhoi_sb[:].rearrange(
                            "p (s m) -> p s m", m=M10)[
                            :, s0:s0 + ssz, m0:m0 + mg]
                        xyp = psxy.tile([P, 1024], fp32, tag="xyp")
                        nc.tensor.matmul(out=xyp[:, 0:cols], lhsT=wxl,
                                         rhs=mov, start=True, stop=True)
                        nc.tensor.matmul(out=xyp[:, 512:512 + cols],
                                         lhsT=wyl, rhs=mov,
                                         start=True, stop=True)
                        xysb = ckpool.tile([P, 1024], bf16, tag="xysb")
                        csrc = xyp[:].rearrange("p (h q) -> p h q", h=2)[
                            :, :, 0:cols]
                        cdst = xysb[:].rearrange("p (h q) -> p h q", h=2)[
                            :, :, 0:cols]
                        if nxcopy % 2 == 0:
                            nc.vector.tensor_copy(out=cdst, in_=csrc)
                        else:
                            nc.scalar.copy(out=cdst, in_=csrc)
                        nxcopy += 1
                        xyt = ckpool.tile([P, 512], fp32, tag="xyt")
                        if mg == 1:
                            nc.vector.tensor_tensor(
                                out=xyt[:, 0:ssz], in0=xysb[:, 0:cols],
                                in1=xysb[:, 512:512 + cols], op=alu.mult)
                        else:
                            txy = ckpool.tile([P, 512], bf16, tag="txy")
                            nc.vector.tensor_tensor(
                                out=txy[:, 0:cols], in0=xysb[:, 0:cols],
                                in1=xysb[:, 512:512 + cols], op=alu.mult)
                            nc.vector.tensor_reduce(
                                out=xyt[:, 0:ssz],
                                in_=txy[:, 0:cols].rearrange(
                                    "p (s m) -> p s m", m=mg),
                                axis=mybir.AxisListType.X, op=alu.add,
                            )
                        nc.sync.dma_start(
                            out=xy_d[:, l * NS + s0:l * NS + s0 + ssz],
                            in_=xyt[:, 0:ssz])

                # extract m=0 plane (stride-10 gather) for the rhoi0 output
                r0t = ckpool.tile([P, slotc], bf16, tag="r0t")
                nc.gpsimd.tensor_copy(
                    out=r0t[:],
                    in_=rhoi_sb[:].rearrange("p (s m) -> p s m", m=M10)[
                        :, base:base + slotc, 0],
                )
                nc.sync.dma_start(out=rhoi0_d[:, base:base + slotc],
                                  in_=r0t[:])

    nc.finalize()
    return nc


# ============================ entry point ============================

def kernel(**inputs):
    from concourse.bass_utils import run_bass_kernel_spmd

    species = np.asarray(inputs["species"], np.int64)
    N_NODES = species.shape[0]
    cores, deg = _partition_and_pack(np.asarray(inputs["edge_src"]), N_NODES)
    maxb = max(len(c["blocks"]) for c in cores)
    nchunk = (maxb + CH - 1) // CH
    B = nchunk * CH
    NS = NSLOT * B

    per_core = _build_host_inputs(inputs, cores, deg, B, nchunk)

    wx = np.empty((P, 3 * NCHAN), np.float32)
    wy = np.empty((P, 3 * NCHAN), np.float32)
    for l, key in enumerate(("W0", "W1", "W2")):
        Wp = _perm_w(inputs[key])
        wx[:, l * NCHAN:(l + 1) * NCHAN] = Wp[:, :NCHAN]
        wy[:, l * NCHAN:(l + 1) * NCHAN] = (
            Wp[:, NCHAN:] / np.sqrt(2 * l + 1.0))
    wx = wx.astype(BF16)
    wy = wy.astype(BF16)

    key = nchunk
    if key not in _COMPILED:
        _COMPILED[key] = _build_program(nchunk)
    nc = _COMPILED[key]

    in_maps = [
        {"edf": pc["edf"], "senc": pc["senc"], "oh": pc["oh"],
         "wx": wx, "wy": wy}
        for pc in per_core
    ]
    res = run_bass_kernel_spmd(nc, in_maps, list(range(NCORES)),
                               trace=TRACE)
    global LAST_RESULT
    LAST_RESULT = res

    # ---------------- host assembly ----------------
    st = np.asarray(inputs["species_table"], np.float32)
    out = np.zeros((N_NODES, N_SPEC + NB + 3 * NCHAN), np.float32)
    out[:, :N_SPEC] = st[species]

    # device basis row of original index rs = r*16+s is dev = s*8+r
    r = np.arange(NB) // N_SPEC
    s = np.arange(NB) % N_SPEC
    dev_of_rs = s * N_RADIAL + r

    for c in range(NCORES):
        sn = per_core[c]["slot_node"]
        valid = sn >= 0
        nodes = sn[valid]
        slots = np.nonzero(valid)[0]
        r0 = np.asarray(res.results[c]["rhoi0"], np.float32)  # [128, NS]
        xy = res.results[c]["xy"]  # [128, 3*NS]
        out[nodes, N_SPEC:N_SPEC + NB] = r0[dev_of_rs][:, slots].T
        for l in range(3):
            out[nodes,
                N_SPEC + NB + l * NCHAN:N_SPEC + NB + (l + 1) * NCHAN] = (
                xy[:, l * NS + slots].T)
    return out


# revision 46
# speedup vs baseline: 1.0346x; 1.0346x over previous
"""Trainium2 Bass kernel for FOAM embedding (GNN message passing).

Strategy (8 NeuronCores, SPMD, no collectives):
  - Edges are sorted by edge_src. Host partitions nodes into 8 contiguous
    ranges with balanced edge counts; each core owns its nodes' edges.
  - Within a core, nodes are packed greedily into "blocks" of <=128 edges
    and <=7 node slots. Each block's 128 edge slots sit on the 128 SBUF
    partitions.
  - The segment-sum over edges becomes one PE matmul per block:
        lhsT = Dij [128e x 128b]   (stationary)
        rhs  = S   [128e x 70]     S[e, l*10+m] = ohw[e,l,m] * Y[e, m]
    where ohw folds the slot one-hot, the SH constants k_m and the
    per-edge switch factor sqrt(2/rc)*switch/d (host-side constants /
    trivial input scalings).  This gives PSUM [128b x (slot, m)] = rhoi
    for up to 7 nodes at once.
  - Phase 3 contracts rhoi with the (row-permuted) Dense weights over the
    128 basis dim on the PE; xl/yl land in one two-bank PSUM tile, one
    copy to SBUF, then a bf16 2x multiply + strided reduce for
    (xl*yl).sum(m).
  - Host reassembles the full [15000, 528] output (species enc columns
    are a pure table gather, done on host).
"""

import os
import sys

import numpy as np

for _p in ("/opt/trn_rl_repo", "/root/.axon_site/_ro/trn_rl_repo"):
    if os.path.isdir(_p) and _p not in sys.path:
        sys.path.insert(0, _p)

import ml_dtypes  # noqa: E402

# ---------------- problem constants (hardcoded per spec) ----------------
N_RADIAL = 8
N_SPEC = 16
ZMAX = 64
CUTOFF = 5.0
NCHAN = 128
NB = N_RADIAL * N_SPEC  # 128 basis
M9 = 9                  # real SH components up to l=2
M10 = 10                # padded (plane 9 is zero)

NCORES = 8
P = 128                 # edges per block == partitions
NSLOT = 7               # node slots per block
SCOLS = NSLOT * M10     # 70 moving columns per block
CH = 56                 # blocks per chunk
PSG = 7                 # blocks per PSUM scatter tile (7*70=490 <= 512)

BF16 = ml_dtypes.bfloat16

_COMPILED = {}
TRACE = False          # set True to capture an NTFF profile
LAST_RESULT = None     # BassKernelResults of the last kernel() call

# internal SH plane order (l-groups contiguous; order within group is free):
#   m0: 1, m1..3: x,y,z, m4: xy, m5: yz, m6: xz, m7: 2z^2-x^2-y^2,
#   m8: x^2-y^2, m9: zero pad
_S5, _S15 = 5.0 ** 0.5, 15.0 ** 0.5
KM = np.array([1.0, 3.0 ** 0.5, 3.0 ** 0.5, 3.0 ** 0.5,
               _S15, _S15, _S15, 0.5 * _S5, 0.5 * _S15, 0.0], np.float32)


# ======================= host-side preprocessing =======================

def _partition_and_pack(edge_src, n_nodes):
    """Split nodes into NCORES contiguous ranges (edge balanced), then pack
    nodes into blocks of <=P edges / <=NSLOT nodes per core."""
    es = np.asarray(edge_src, dtype=np.int64)
    E = es.shape[0]
    deg = np.bincount(es, minlength=n_nodes)
    splits = [0]
    for c in range(1, NCORES):
        n = int(es[min((c * E) // NCORES, E - 1)])
        n = max(n, splits[-1])
        splits.append(n)
    splits.append(n_nodes)

    cores = []
    for c in range(NCORES):
        nlo, nhi = splits[c], splits[c + 1]
        blocks = []
        n = nlo
        while n < nhi:
            cnt = 0
            esum = 0
            while (n + cnt < nhi and cnt < NSLOT
                   and esum + deg[n + cnt] <= P):
                esum += deg[n + cnt]
                cnt += 1
            if cnt == 0:
                raise ValueError(
                    f"node {n} has degree {deg[n]} > {P}; unsupported")
            blocks.append((n, cnt, esum))
            n += cnt
        cores.append({"nlo": nlo, "nhi": nhi, "blocks": blocks})
    return cores, deg


def _build_host_inputs(inputs, cores, deg, B, nchunk):
    """Build per-core DRAM input arrays in the device layout."""
    dist = np.asarray(inputs["distances"], np.float32)
    vec = np.asarray(inputs["vec"], np.float32)
    switch = np.asarray(inputs["switch"], np.float32)
    st = np.asarray(inputs["species_table"], np.float32)
    species = np.asarray(inputs["species"], np.int64)
    esrc = np.asarray(inputs["edge_src"], np.int64)
    edst = np.asarray(inputs["edge_dst"], np.int64)
    N_NODES = species.shape[0]

    senc_node = st[species]          # [N, 16]
    first_edge = np.searchsorted(esrc, np.arange(N_NODES + 1), side="left")
    bess = (2.0 / CUTOFF) ** 0.5
    swf = bess * switch / dist       # per-edge switch factor (folded in ohw)

    per_core = []
    for c in range(NCORES):
        blocks = cores[c]["blocks"]
        nb = len(blocks)
        edf = np.zeros((B, 4, P), np.float32)
        edf[:, 0, :] = 1.0                      # dist pad
        edf[:, 1, :] = 1.0                      # vx pad
        senc_e = np.zeros((B, P, N_SPEC), np.float32)
        ohw = np.zeros((B, P, NSLOT), np.float32)
        slot_node = np.full((B * NSLOT,), -1, np.int64)

        for k, (n0, cnt, esum) in enumerate(blocks):
            e0 = first_edge[n0]
            e1 = first_edge[n0 + cnt]
            idx = np.arange(e0, e1)
            p = idx - e0
            edf[k, 0, p] = dist[idx]
            edf[k, 1, p] = vec[idx, 0]
            edf[k, 2, p] = vec[idx, 1]
            edf[k, 3, p] = vec[idx, 2]
            senc_e[k, p, :] = senc_node[edst[idx]]
            loc = esrc[idx] - n0
            ohw[k, p, loc] = swf[idx]
            slot_node[k * NSLOT: k * NSLOT + cnt] = np.arange(n0, n0 + cnt)

        # edf planes, whole core: [128, 4, B]
        edf_dev = np.ascontiguousarray(edf.transpose(2, 1, 0))
        # senc_rep[p, c, s, r] = senc[p, c, s]
        senc_dev = np.ascontiguousarray(
            np.repeat(
                senc_e.reshape(nchunk, CH, P, N_SPEC).transpose(0, 2, 1, 3),
                N_RADIAL, axis=3,
            )
        ).astype(BF16)
        # ohw[p, c, l] = oh[p, c, l] * swf[e]   (km folded into Y on device)
        oh_dev = np.ascontiguousarray(
            ohw.reshape(nchunk, CH, P, NSLOT).transpose(0, 2, 1, 3)
        ).astype(BF16)

        per_core.append(
            {
                "edf": edf_dev.reshape(P, 4 * B),
                "senc": senc_dev.reshape(nchunk, P, CH * NB),
                "oh": oh_dev.reshape(nchunk, P, CH * NSLOT),
                "slot_node": slot_node,
                "nblocks": nb,
            }
        )
    return per_core


def _perm_w(W):
    """Permute Dense weight rows from rs-order (r*16+s) to (s*8+r) order."""
    W = np.asarray(W, np.float32)
    return np.ascontiguousarray(
        W.reshape(N_RADIAL, N_SPEC, -1).transpose(1, 0, 2).reshape(NB, -1)
    )


# ========================= device program =========================

def _build_program(nchunk):
    import concourse.bacc as bacc
    import concourse.mybir as mybir
    import concourse.tile as tile
    from concourse.alu_op_type import AluOpType as alu

    fp32 = mybir.dt.float32
    bf16 = mybir.dt.bfloat16

    B = nchunk * CH
    NS = NSLOT * B

    nc = bacc.Bacc("TRN2", target_bir_lowering=False, debug=False,
                   num_devices=NCORES)

    edf_d = nc.dram_tensor("edf", [P, 4 * B], fp32, kind="ExternalInput")
    senc_d = nc.dram_tensor("senc", [nchunk, P, CH * NB], bf16,
                            kind="ExternalInput")
    oh_d = nc.dram_tensor("oh", [nchunk, P, CH * NSLOT], bf16,
                          kind="ExternalInput")
    wx_d = nc.dram_tensor("wx", [P, 3 * NCHAN], bf16, kind="ExternalInput")
    wy_d = nc.dram_tensor("wy", [P, 3 * NCHAN], bf16, kind="ExternalInput")
    rhoi0_d = nc.dram_tensor("rhoi0", [P, NS], bf16, kind="ExternalOutput")
    xy_d = nc.dram_tensor("xy", [P, 3 * NS], fp32, kind="ExternalOutput")

    with tile.TileContext(nc) as tc:
        with (
            tc.tile_pool(name="const", bufs=1) as cpool,
            tc.tile_pool(name="pha", bufs=1) as papool,
            tc.tile_pool(name="chunk", bufs=2) as ckpool,
            tc.tile_pool(name="big", bufs=1) as bigpool,
            tc.tile_pool(name="ps_sc", bufs=4, space="PSUM") as pssc,
            tc.tile_pool(name="ps_xy", bufs=2, space="PSUM") as psxy,
        ):
            wx = cpool.tile([P, 3 * NCHAN], bf16, tag="wx")
            wy = cpool.tile([P, 3 * NCHAN], bf16, tag="wy")
            nc.sync.dma_start(out=wx[:], in_=wx_d[:])
            nc.sync.dma_start(out=wy[:], in_=wy_d[:])
            half_pi = cpool.tile([P, 1], fp32, tag="halfpi")
            nc.vector.memset(half_pi[:], float(np.pi / 2))

            rhoi_sb = bigpool.tile([P, M10 * NS], bf16, tag="rhoi")

            # ============ phase A: per-edge scalars, whole core ============
            edf = papool.tile([P, 4 * B], fp32, tag="edf")
            nc.sync.dma_start(out=edf[:], in_=edf_d[:])
            d_ap = edf[:, 0:B]
            v_ap = edf[:, B:4 * B]

            rinv = papool.tile([P, B], fp32, tag="rinv")
            nc.vector.reciprocal(out=rinv[:], in_=d_ap)
            u = papool.tile([P, 3 * B], fp32, tag="u")
            nc.vector.tensor_tensor(
                out=u[:].rearrange("p (t c) -> p t c", t=3),
                in0=v_ap.rearrange("p (t c) -> p t c", t=3),
                in1=rinv[:].unsqueeze(1).broadcast_to([P, 3, B]),
                op=alu.mult,
            )
            ux, uy, uz = (u[:, i * B:(i + 1) * B] for i in range(3))

            # radial: rbp[p, n, c] = sin((n+1) theta), theta = pi d / rc,
            # via Chebyshev recurrence (ACT Sin valid on [-4.18, 4.18]).
            # Built on contiguous n-major planes (strided writes are slow),
            # then one transposing cast to r-innermost bf16.
            rbp = papool.tile([P, N_RADIAL * B], fp32, tag="rbp")
            cos2 = papool.tile([P, B], fp32, tag="cos2")
            nc.scalar.activation(
                out=rbp[:, 0:B], in_=d_ap,
                func=mybir.ActivationFunctionType.Sin,
                scale=float(np.pi / CUTOFF),
            )
            nc.scalar.activation(
                out=cos2[:], in_=d_ap,
                func=mybir.ActivationFunctionType.Sin,
                scale=float(-np.pi / CUTOFF), bias=half_pi[:],
            )
            nc.vector.tensor_scalar(
                out=cos2[:], in0=cos2[:], scalar1=2.0, scalar2=None,
                op0=alu.mult,
            )
            nc.vector.tensor_tensor(
                out=rbp[:, B:2 * B], in0=cos2[:], in1=rbp[:, 0:B],
                op=alu.mult)
            for n in range(2, N_RADIAL):
                nc.vector.tensor_tensor(
                    out=rbp[:, n * B:(n + 1) * B], in0=cos2[:],
                    in1=rbp[:, (n - 1) * B:n * B], op=alu.mult)
                nc.vector.tensor_tensor(
                    out=rbp[:, n * B:(n + 1) * B],
                    in0=rbp[:, n * B:(n + 1) * B],
                    in1=rbp[:, (n - 2) * B:(n - 1) * B], op=alu.subtract)
            rb_t = papool.tile([P, B * N_RADIAL], bf16, tag="rbt")
            nc.vector.tensor_copy(
                out=rb_t[:].rearrange("p (c n) -> p c n", n=N_RADIAL),
                in_=rbp[:].rearrange("p (n c) -> p n c", n=N_RADIAL)
                    .transpose([0, 2, 1]),
            )

            # Y planes, m-major fp32 (contiguous builds), then one
            # transposing cast to m-innermost bf16 for the S broadcast.
            Yp = papool.tile([P, M10 * B], fp32, tag="Yp")
            nc.vector.memset(Yp[:, 0:B], 1.0)
            nc.vector.memset(Yp[:, 9 * B:10 * B], 0.0)
            nc.vector.tensor_copy(out=Yp[:, B:4 * B], in_=u[:])
            # m4 = x*y, m5 = y*z  (pair op), m6 = x*z
            nc.vector.tensor_tensor(
                out=Yp[:, 4 * B:6 * B], in0=u[:, 0:2 * B],
                in1=u[:, B:3 * B], op=alu.mult)
            nc.vector.tensor_tensor(
                out=Yp[:, 6 * B:7 * B], in0=ux, in1=uz, op=alu.mult)
            # m7 = 2 z^2 - x^2 - y^2, m8 = x^2 - y^2   (|u| = 1)
            sq = papool.tile([P, 3 * B], fp32, tag="sq")
            nc.vector.tensor_tensor(out=sq[:], in0=u[:], in1=u[:],
                                    op=alu.mult)
            ab = papool.tile([P, B], fp32, tag="ab")
            nc.vector.tensor_tensor(
                out=ab[:], in0=sq[:, 0:B], in1=sq[:, B:2 * B], op=alu.add)
            nc.vector.scalar_tensor_tensor(
                out=Yp[:, 7 * B:8 * B], in0=sq[:, 2 * B:3 * B], scalar=2.0,
                in1=ab[:], op0=alu.mult, op1=alu.subtract)
            nc.vector.tensor_tensor(
                out=Yp[:, 8 * B:9 * B], in0=sq[:, 0:B], in1=sq[:, B:2 * B],
                op=alu.subtract)
            # transpose-cast to m-inner, folding the SH constants km in:
            # Y[p, c, m] = Yp[p, m, c] * km[m]
            kmt = cpool.tile([P, M10], fp32, tag="kmt")
            for m in range(M10):
                nc.vector.memset(kmt[:, m:m + 1], float(KM[m]))
            Y = papool.tile([P, B * M10], bf16, tag="Y")
            nc.vector.tensor_tensor(
                out=Y[:].rearrange("p (c m) -> p c m", m=M10),
                in0=Yp[:].rearrange("p (m c) -> p m c", m=M10)
                    .transpose([0, 2, 1]),
                in1=kmt[:].unsqueeze(1).broadcast_to([P, B, M10]),
                op=alu.mult,
            )

            # ================= per-chunk scatter + phase 3 =================
            ncopy = 0
            nxcopy = 0
            for ci in range(nchunk):
                senc = ckpool.tile([P, CH * NB], bf16, tag="senc")
                oh = ckpool.tile([P, CH * NSLOT], bf16, tag="oh")
                nc.sync.dma_start(out=senc[:], in_=senc_d[ci])
                nc.sync.dma_start(out=oh[:], in_=oh_d[ci])

                # S[p, blk, l*10+m] = ohw[p, blk, l] * Y[p, blk, m]
                S = ckpool.tile([P, CH * SCOLS], bf16, tag="S")
                nc.gpsimd.tensor_tensor(
                    out=S[:].rearrange("p (c l m) -> p c l m",
                                       l=NSLOT, m=M10),
                    in0=oh[:].rearrange("p (c l) -> p c l", l=NSLOT)
                        .unsqueeze(3).broadcast_to([P, CH, NSLOT, M10]),
                    in1=Y[:, ci * CH * M10:(ci + 1) * CH * M10]
                        .rearrange("p (c m) -> p c m", m=M10)
                        .unsqueeze(2).broadcast_to([P, CH, NSLOT, M10]),
                    op=alu.mult,
                )
                # Dij[p, blk, s*8+r] = senc_rep[p, blk, s, r] * rb_t[p, blk, r]
                Dij = ckpool.tile([P, CH * NB], bf16, tag="Dij")
                nc.vector.tensor_tensor(
                    out=Dij[:].rearrange("p (c s r) -> p c s r",
                                         s=N_SPEC, r=N_RADIAL),
                    in0=senc[:].rearrange("p (c s r) -> p c s r",
                                          s=N_SPEC, r=N_RADIAL),
                    in1=rb_t[:, ci * CH * N_RADIAL:(ci + 1) * CH * N_RADIAL]
                        .rearrange("p (c r) -> p c r", r=N_RADIAL)
                        .unsqueeze(2).broadcast_to([P, CH, N_SPEC, N_RADIAL]),
                    op=alu.mult,
                )

                # scatter matmuls: PSG blocks per PSUM tile, then one
                # contiguous copy into slot-major rhoi_sb (col = slot*10+m)
                for g in range(CH // PSG):
                    pst = pssc.tile([P, PSG * SCOLS], fp32, tag="psc")
                    for j in range(PSG):
                        k = g * PSG + j
                        nc.tensor.matmul(
                            out=pst[:, j * SCOLS:(j + 1) * SCOLS],
                            lhsT=Dij[:, k * NB:(k + 1) * NB],
                            rhs=S[:, k * SCOLS:(k + 1) * SCOLS],
                            start=True, stop=True,
                        )
                    col0 = (ci * CH + g * PSG) * NSLOT * M10
                    dst = rhoi_sb[:, col0:col0 + PSG * SCOLS]
                    nc.scalar.copy(out=dst, in_=pst[:])
                    ncopy += 1

                # ---- phase 3, interleaved per chunk ----
                slotc = CH * NSLOT
                base = ci * slotc
                for l in range(3):
                    mg = 2 * l + 1
                    m0 = l * l
                    nsl = -(-slotc // (512 // mg))
                    ssz0 = -(-slotc // nsl)
                    wxl = wx[:, l * NCHAN:(l + 1) * NCHAN]
                    wyl = wy[:, l * NCHAN:(l + 1) * NCHAN]
                    for t in range(nsl):
                        s0 = base + t * ssz0
                        ssz = min(ssz0, base + slotc - s0)
                        cols = ssz * mg
                        mov = rhoi_sb[:].rearrange(
                            "p (s m) -> p s m", m=M10)[
                            :, s0:s0 + ssz, m0:m0 + mg]
                        xyp = psxy.tile([P, 1024], fp32, tag="xyp")
                        nc.tensor.matmul(out=xyp[:, 0:cols], lhsT=wxl,
                                         rhs=mov, start=True, stop=True)
                        nc.tensor.matmul(out=xyp[:, 512:512 + cols],
                                         lhsT=wyl, rhs=mov,
                                         start=True, stop=True)
                        xysb = ckpool.tile([P, 1024], bf16, tag="xysb")
                        csrc = xyp[:].rearrange("p (h q) -> p h q", h=2)[
                            :, :, 0:cols]
                        cdst = xysb[:].rearrange("p (h q) -> p h q", h=2)[
                            :, :, 0:cols]
                        if nxcopy % 4 == 3:
                            nc.vector.tensor_copy(out=cdst, in_=csrc)
                        else:
                            nc.scalar.copy(out=cdst, in_=csrc)
                        nxcopy += 1
                        xyt = ckpool.tile([P, 512], fp32, tag="xyt")
                        if mg == 1:
                            nc.vector.tensor_tensor(
                                out=xyt[:, 0:ssz], in0=xysb[:, 0:cols],
                                in1=xysb[:, 512:512 + cols], op=alu.mult)
                        else:
                            txy = ckpool.tile([P, 512], bf16, tag="txy")
                            nc.vector.tensor_tensor(
                                out=txy[:, 0:cols], in0=xysb[:, 0:cols],
                                in1=xysb[:, 512:512 + cols], op=alu.mult)
                            nc.vector.tensor_reduce(
                                out=xyt[:, 0:ssz],
                                in_=txy[:, 0:cols].rearrange(
                                    "p (s m) -> p s m", m=mg),
                                axis=mybir.AxisListType.X, op=alu.add,
                            )
                        nc.sync.dma_start(
                            out=xy_d[:, l * NS + s0:l * NS + s0 + ssz],
                            in_=xyt[:, 0:ssz])

                # extract m=0 plane (stride-10 gather) for the rhoi0 output
                r0t = ckpool.tile([P, slotc], bf16, tag="r0t")
                nc.gpsimd.tensor_copy(
                    out=r0t[:],
                    in_=rhoi_sb[:].rearrange("p (s m) -> p s m", m=M10)[
                        :, base:base + slotc, 0],
                )
                nc.sync.dma_start(out=rhoi0_d[:, base:base + slotc],
                                  in_=r0t[:])

    nc.finalize()
    return nc


# ============================ entry point ============================

def kernel(**inputs):
    from concourse.bass_utils import run_bass_kernel_spmd

    species = np.asarray(inputs["species"], np.int64)
    N_NODES = species.shape[0]
    cores, deg = _partition_and_pack(np.asarray(inputs["edge_src"]), N_NODES)
    maxb = max(len(c["blocks"]) for c in cores)
    nchunk = (maxb + CH - 1) // CH
    B = nchunk * CH
    NS = NSLOT * B

    per_core = _build_host_inputs(inputs, cores, deg, B, nchunk)

    wx = np.empty((P, 3 * NCHAN), np.float32)
    wy = np.empty((P, 3 * NCHAN), np.float32)
    for l, key in enumerate(("W0", "W1", "W2")):
        Wp = _perm_w(inputs[key])
        wx[:, l * NCHAN:(l + 1) * NCHAN] = Wp[:, :NCHAN]
        wy[:, l * NCHAN:(l + 1) * NCHAN] = (
            Wp[:, NCHAN:] / np.sqrt(2 * l + 1.0))
    wx = wx.astype(BF16)
    wy = wy.astype(BF16)

    key = nchunk
    if key not in _COMPILED:
        _COMPILED[key] = _build_program(nchunk)
    nc = _COMPILED[key]

    in_maps = [
        {"edf": pc["edf"], "senc": pc["senc"], "oh": pc["oh"],
         "wx": wx, "wy": wy}
        for pc in per_core
    ]
    res = run_bass_kernel_spmd(nc, in_maps, list(range(NCORES)),
                               trace=TRACE)
    global LAST_RESULT
    LAST_RESULT = res

    # ---------------- host assembly ----------------
    st = np.asarray(inputs["species_table"], np.float32)
    out = np.zeros((N_NODES, N_SPEC + NB + 3 * NCHAN), np.float32)
    out[:, :N_SPEC] = st[species]

    # device basis row of original index rs = r*16+s is dev = s*8+r
    r = np.arange(NB) // N_SPEC
    s = np.arange(NB) % N_SPEC
    dev_of_rs = s * N_RADIAL + r

    for c in range(NCORES):
        sn = per_core[c]["slot_node"]
        valid = sn >= 0
        nodes = sn[valid]
        slots = np.nonzero(valid)[0]
        r0 = np.asarray(res.results[c]["rhoi0"], np.float32)  # [128, NS]
        xy = res.results[c]["xy"]  # [128, 3*NS]
        out[nodes, N_SPEC:N_SPEC + NB] = r0[dev_of_rs][:, slots].T
        for l in range(3):
            out[nodes,
                N_SPEC + NB + l * NCHAN:N_SPEC + NB + (l + 1) * NCHAN] = (
                xy[:, l * NS + slots].T)
    return out
